# revision 48
# baseline (speedup 1.0000x reference)
"""Multi-head self-attention Trainium2 Bass kernel (8-core SPMD).

Sharding: tensor-parallel over (batch, head-pair). With B=2 batches and
H=8 heads there are exactly 8 (batch, head-pair) units; core c handles
batch c//4 and heads {2*(c%4), 2*(c%4)+1}. Each core computes Q/K/V for its
two heads over the full sequence, runs attention, and produces the partial
output projection O_pair @ Wo_pair (no bias). The host sums the four
partials per batch and adds the output bias — a cheap numpy reduction.
Per-core weight slices are passed as separate inputs so the program stays
SPMD-uniform.

Layout strategy: activations live transposed in SBUF ([D, S], d on
partitions). Projections then need no weight transposes:
  K^T = Wk^T x^T   (lhsT = Wk chunk, rhs = x^T chunk)
  V   = x Wv       (lhsT = x^T chunk, rhs = Wv chunk)
Scores are computed transposed ([k, q], k on partitions) so softmax's
denominator comes from a ones-column appended to V (row 64 of the attention
output accumulator), and A^T is directly consumable by the A@V matmul.
exp() runs on the scalar engine with the 1/sqrt(dk) folded into its scale.
The normalized per-head outputs O^T are exactly the lhsT the output
projection wants, so no transposes are needed anywhere except on the input x.

Matmul operands are fp16 except A@V, which runs in fp8e4 with
perf_mode=DoubleRow: exp() writes E directly as e4m3 in a k-tile-pair
layout and each A@V matmul consumes the pair at 2 fp8 MACs per PE cell
(measured end-to-end absmax relative error ~9e-3 vs the 2e-2 gate). All
accumulation is fp32 in PSUM.

Scheduling is built around two facts about the machine: engines execute
their queues in order, and the scalar engine's exp() stream (~266us
busy) is the pacing resource. So (1) the per-qc normalize/out-projection
work is deferred into the NEXT qc's k-tile loop and pinned there with
scheduler dependency edges — at the point where the PE would idle
waiting for exp to free a score PSUM slot; (2) the projections of
sequence halves 2-7 are prefetched (DMA + DVE f16 cast) and emitted as
~1us chunks paced through qc0's k-tile loop the same way; (3) the two
heads' score matmuls are row-tiled (tile_position row strips 0-63 /
64-127) so each k-tile's pair streams concurrently.
"""

from contextlib import ExitStack

import numpy as np

import concourse.bass as bass
import concourse.tile as tile
from concourse import bacc, mybir
from concourse.bass import _add_dep_helper
from concourse.bass_utils import run_bass_kernel_spmd

N_CORES = 8
B, S, D, H, DK = 2, 4096, 512, 8, 64
P = 128
NT_S = S // P                  # 32 sequence tiles
NT_D = D // P                  # 4 d-model chunks
QC = S // 512                  # 8 query chunks of 512
VW = 2 * 65                    # 130: per-k-tile width of the augmented V
F32 = mybir.dt.float32
F32R = mybir.dt.float32r
F16 = mybir.dt.float16
F8 = mybir.dt.float8e4
DR = mybir.MatmulPerfMode.DoubleRow
EXP = mybir.ActivationFunctionType.Exp

# "f16" (10 mantissa bits, 2.4 GHz MAC path + FWL), "f32r" (13 bits but
# pinned at the 1.2 GHz throttled clock), "f32" (exact, 4 cycles/row).
MM_DTYPE = "f16"
DTM = {"f32r": F32R, "f16": F16, "f32": F32}[MM_DTYPE]

# exp(x) ~= C[0]x^5 + ... on [-2.5, 2.5] (least-squares at Chebyshev
# nodes; logits are N(0, 0.33), observed |max| ~1.9). Used by the DVE
# polynomial-exp offload; abs err ~0.018 in f16 — far below the fp8e4
# quantization the result is stored at.
EC5, EC4, EC3, EC2, EC1, EC0 = (0.010762342, 0.056516835, 0.15879715,
                                0.46371029, 1.0062593, 1.0128646)
MULT = mybir.AluOpType.mult
ADD = mybir.AluOpType.add
# which k-tiles of each qc run exp on DVE instead of the scalar engine.
# Measured: DVE runs these ops at 1x (no 2x/4x modes engage) so the
# 7-pass polynomial costs ~7.5us per tile vs ACT's 1.14us — offload is a
# net loss; kept as an empty hook.
DVE_EXP_KT = {_q: () for _q in range(8)}


def _emit(ctx: ExitStack, tc: tile.TileContext, io: dict):
    nc = tc.nc
    xb = io["xb"]
    wqp, wkp, wvp, wop = io["wqp"], io["wkp"], io["wvp"], io["wop"]
    bqp, bkp, bvp = io["bqp"], io["bkp"], io["bvp"]
    ident = io["ident"]
    out = io["out"]

    mm = nc.tensor.matmul

    # ---- pools ------------------------------------------------------------
    consts = ctx.enter_context(tc.tile_pool(name="consts", bufs=1))
    xt_pool = ctx.enter_context(tc.tile_pool(name="xt", bufs=1))
    qt_pool = ctx.enter_context(tc.tile_pool(name="qt", bufs=1))
    kt_pool = ctx.enter_context(tc.tile_pool(name="kt", bufs=1))
    v_pool = ctx.enter_context(tc.tile_pool(name="v", bufs=1))
    ot_pool = ctx.enter_context(tc.tile_pool(name="ot", bufs=2))
    w_pool = ctx.enter_context(tc.tile_pool(name="w", bufs=1))
    stg = ctx.enter_context(tc.tile_pool(name="stg", bufs=3))
    e_pool = ctx.enter_context(tc.tile_pool(name="e", bufs=8))
    rc_pool = ctx.enter_context(tc.tile_pool(name="rc", bufs=4))
    y_pool = ctx.enter_context(tc.tile_pool(name="y", bufs=3))
    # PSUM (8 banks): shared scores/normalize/out-proj ring [128,1024]x3
    # = 6 banks + attention accumulators [65,512]x2 = 2 banks. The ring
    # of 3 gives the score stream one extra tile of exp lookahead; the
    # few normalize/proj allocs per qc ride the same ring (safe now that
    # they are deferred+pinned past the qc boundary).
    ps_pool = ctx.enter_context(tc.tile_pool(name="ps", bufs=2, space="PSUM"))
    o_pool = ctx.enter_context(tc.tile_pool(name="o", bufs=4, space="PSUM"))

    def psum1024(dt=F32):
        return ps_pool.tile([P, 1024], dt, tag="ps", name="ps")

    def psum512(dt=F32):
        return psum1024(dt)[:, 0:512]

    # ---- constants --------------------------------------------------------
    ident_sb = consts.tile([P, P], F32, tag="ident")
    nc.scalar.dma_start(out=ident_sb[:], in_=ident[:])
    ident16 = consts.tile([P, P], F16, tag="ident16")
    nc.vector.tensor_copy(out=ident16[:], in_=ident_sb[:])
    ones_f32 = consts.tile([P, 1], F32, tag="ones_f32")
    nc.vector.memset(ones_f32[:], 1.0)
    ones_sb = consts.tile([1, P], DTM, tag="ones")
    nc.vector.tensor_copy(out=ones_sb[:], in_=ones_f32[0:1, 0:1].broadcast_to([1, P]))
    # a f16 ones row living on partition 64 (denominator broadcast lhsT)
    ones64_sb = consts.tile([65, 64], F16, tag="ones64")
    nc.vector.memset(ones64_sb[64:65, :], 1.0)
    # per-partition bias columns for K^T/Q^T (fused into the PSUM->SBUF
    # copies); bv as a [1, 128] row for the rank-1 bias matmul.
    bkT = consts.tile([P, 1], F32, tag="bkT")
    nc.scalar.dma_start(out=bkT[:], in_=bkp[:])
    bqT = consts.tile([P, 1], F32, tag="bqT")
    nc.scalar.dma_start(out=bqT[:], in_=bqp[:])
    bv_st = consts.tile([1, P], F32, tag="bv_st")
    nc.scalar.dma_start(out=bv_st[:], in_=bvp[:])
    bv_sb = consts.tile([1, P], DTM, tag="bv")
    nc.vector.tensor_copy(out=bv_sb[:], in_=bv_st[:])

    # per-core weight slices -> fp16 SBUF tiles
    def load_w(ap, rows, cols, tag):
        st = stg.tile([P, (rows // P) * cols], F32, tag="wstg")
        nc.sync.dma_start(
            out=st[:, :].rearrange("p (dc m) -> p dc m", dc=rows // P),
            in_=ap.rearrange("(dc p) m -> p dc m", p=P),
        )
        t = w_pool.tile([P, (rows // P) * cols], DTM, tag=tag)
        nc.vector.tensor_copy(out=t[:], in_=st[:])
        return t

    # x^T, Q^T, K^T, V are held at sequence-HALF-CHUNK granularity (8
    # tiles of 512 sequence positions each) so dependency tracking lets
    # attention start as soon as the first 512-chunk of K/Q/V exists, and
    # the remaining projection work streams into qc0's slack in ~1us
    # chunks without ever starving the scalar engine.
    SH = 512                    # sequence columns per half-chunk
    xTh = [xt_pool.tile([P, NT_D * SH], DTM, tag="xT", name=f"xT{j}",
                        bufs=8) for j in range(8)]

    def xslice(dc, s0, s1):
        j = s0 // SH
        return xTh[j][:, dc * SH + s0 - j * SH: dc * SH + s1 - j * SH]

    # ---- stages A+B, emitted as fine-grained chunks ----------------------
    wsb = {}
    qth = [qt_pool.tile([P, SH], DTM, tag="QT", name=f"QT{j}", bufs=8)
           for j in range(8)]
    kth = [kt_pool.tile([P, SH], DTM, tag="KT", name=f"KT{j}", bufs=8)
           for j in range(8)]
    # V in fp8e4, packed for DoubleRow A@V: per k-tile PAIR pr and head h,
    # lhsT cols pr%2*320 + h*160 + (ko*80 + j) with ko in {0,1} the two
    # k-tiles of the pair, j<65 (64 V dims + ones column), 15 cols pad so
    # the Ko step (80) is 16-aligned as DoubleRow requires.
    vqh = [v_pool.tile([P, 2 * 320], F8, tag="vaug", name=f"vq{j}", bufs=8)
           for j in range(8)]

    def v_lhsT(pair, h):
        base = (pair % 2) * 320 + h * 160
        return vqh[pair // 2][:, base:base + 160].rearrange(
            "p (ko w) -> p ko w", ko=2)[:, :, 0:65]
    xn_pool = ctx.enter_context(tc.tile_pool(name="xn", bufs=8))
    xh_pool = ctx.enter_context(tc.tile_pool(name="xh", bufs=28))
    xp_pool = ctx.enter_context(tc.tile_pool(name="xp", bufs=4))
    xh_all = {}  # st -> prefetched f16 x tile (quarters 1-3)

    def emit_exp_dve(sp, eat, parity):
        # exp via degree-5 polynomial on the vector engine: one
        # psum-read/scale pass, Horner-style chain in f16 (the
        # scalar_tensor_tensor form (p+c)*x carries no constant term, so
        # the power coefficients map onto the chain constants directly),
        # final +C0 pass writing the fp8 pair-layout slice.
        x16 = xp_pool.tile([P, 1024], F16, tag="x16")
        nc.vector.tensor_scalar(out=x16[:], in0=sp[:], scalar1=0.125,
                                scalar2=None, op0=MULT)
        p = xp_pool.tile([P, 1024], F16, tag="pp")
        nc.vector.tensor_scalar(out=p[:], in0=x16[:], scalar1=EC5,
                                scalar2=EC4, op0=MULT, op1=ADD)
        for g in (0.0, EC3, EC2, EC1):
            p2 = xp_pool.tile([P, 1024], F16, tag="pp")
            nc.vector.scalar_tensor_tensor(out=p2[:], in0=p[:], scalar=g,
                                           in1=x16[:], op0=ADD, op1=MULT)
            p = p2
        nc.vector.tensor_scalar(
            out=eat[:, parity * 1024:(parity + 1) * 1024], in0=p[:],
            scalar1=EC0, scalar2=None, op0=ADD)

    def prefetch_x(j):
        # DMA + f32->f16 cast well ahead of the half-chunk's compute, so
        # the PE work inserted into qc0's score stream never waits on the
        # memory chain. Casts go on DVE: gpsimd CAST measures ~1.9us per
        # tile (3.6x the cost model), DVE does it in ~0.55us.
        for st in range(4 * j, 4 * j + 4):
            xn = xn_pool.tile([P, D], F32, tag="xn")
            nc.sync.dma_start(out=xn[:], in_=xb[st * P:(st + 1) * P, :])
            xh = xh_pool.tile([P, D], F16, tag="xh")
            nc.vector.tensor_copy(out=xh[:], in_=xn[:])
            xh_all[st] = xh

    def pin_first(ins_list, gate):
        if gate is not None and ins_list:
            _add_dep_helper(ins_list[0].ins, gate.ins, sync=False,
                            reason="chunk after scores")

    def half_tr(j, ts, gate=None):
        # transpose x s-tiles 4j+2ts, 4j+2ts+1 into xTh[j]
        first = []
        for st in (4 * j + 2 * ts, 4 * j + 2 * ts + 1):
            if j < 1:
                # prologue: the PE is idle while the first x tiles stream
                # in, so burn the cheap-to-hide f32 transpose (no cast in
                # the latency chain)
                xn = xn_pool.tile([P, D], F32, tag="xn")
                nc.sync.dma_start(out=xn[:], in_=xb[st * P:(st + 1) * P, :])
                tp = psum1024()
                for dc in range(NT_D):
                    nc.tensor.transpose(
                        tp[:, dc * P:(dc + 1) * P],
                        xn[:, dc * P:(dc + 1) * P],
                        ident_sb[:],
                    )
            else:
                # f16 transpose runs at 1 cyc/row + FWL (f32: 2 cyc, none)
                xh = xh_all.pop(st)
                tp = psum1024(F16)
                for dc in range(NT_D):
                    t_i = nc.tensor.transpose(
                        tp[:, dc * P:(dc + 1) * P],
                        xh[:, dc * P:(dc + 1) * P],
                        ident16[:],
                    )
                    if not first:
                        first.append(t_i)
                        pin_first(first, gate)
            dst_ap = xTh[j][:, :].rearrange("p (dc s) -> p dc s", dc=NT_D)
            so = (st % 4) * P
            nc.vector.tensor_copy(
                out=dst_ap[:, :, so:so + P],
                in_=tp[:, 0:512].rearrange("p (dc j) -> p dc j", dc=NT_D),
            )


    def half_kq(j, which, gate=None):
        w_sb, dst, bT = ((wsb["wk"], kth[j], bkT) if which == "k" else
                         (wsb["wq"], qth[j], bqT))
        ps = psum1024()
        for dc in range(NT_D):
            m_i = mm(ps[:, 0:512], w_sb[:, dc * P:(dc + 1) * P],
                     xslice(dc, j * SH, (j + 1) * SH),
                     start=(dc == 0), stop=(dc == NT_D - 1))
            if dc == 0:
                pin_first([m_i], gate)
        nc.vector.tensor_scalar_add(out=dst[:, :], in0=ps[:, 0:512],
                                    scalar1=bT[:])

    def half_v(j, gate=None):
        nc.vector.tensor_copy(
            out=vqh[j][:, :].rearrange("p (pr h ko w) -> p pr h ko w",
                                       pr=2, h=2, ko=2)[:, :, :, :, 64:65],
            in_=ones_f32[:, 0:1].broadcast_to([P, 2, 2, 2, 1]),
        )
        first = []
        for pr in (2 * j, 2 * j + 1):
            # two V s-tiles (= one DoubleRow k-tile pair) per [128,1024]
            # tile (banks 0 and 1)
            ps = psum1024()
            for jj in range(2):
                st = 2 * pr + jj
                for dc in range(NT_D):
                    m_i = mm(ps[:, jj * 512:jj * 512 + P],
                             xslice(dc, st * P, (st + 1) * P),
                             wsb["wv"][:, dc * P:(dc + 1) * P],
                             start=(dc == 0), stop=False)
                    if not first:
                        first.append(m_i)
                        pin_first(first, gate)
                mm(ps[:, jj * 512:jj * 512 + P], ones_sb[0:1, :],
                   bv_sb[0:1, :], start=False, stop=True)
            dst = vqh[j][:, (pr % 2) * 320:(pr % 2 + 1) * 320]
            dst = dst.rearrange("p (h ko w) -> p h ko w", h=2, ko=2)[:, :, :, 0:64]
            src = ps[:, :].rearrange("p (ko r) -> p ko r", ko=2)[:, :, 0:P]
            nc.vector.tensor_copy(
                out=dst, in_=src.rearrange("p ko (h e) -> p h ko e", h=2)
            )

    # halves 0-1 (k-tiles 0-7) up front; halves 2-7 as ~1us chunks paced
    # through qc0's k-tile loop, each finishing before the k-tiles that
    # need it. Weight DMAs are interleaved after the x tiles they'd
    # otherwise delay; each is needed only once its projection starts.
    half_tr(0, 0)
    wsb["wk"] = load_w(wkp, D, P, "wk")
    half_tr(0, 1)
    wsb["wq"] = load_w(wqp, D, P, "wq")
    half_kq(0, "k")
    wsb["wv"] = load_w(wvp, D, P, "wv")
    half_kq(0, "q")
    half_v(0)
    for j in range(1, 8):
        prefetch_x(j)
    half_tr(1, 0)
    half_tr(1, 1)
    half_kq(1, "k")
    half_kq(1, "q")
    half_v(1)
    chunk_at = {}
    chunk_q = []
    for j in range(2, 8):
        chunk_q += [lambda g, j=j: half_tr(j, 0, g),
                    lambda g, j=j: half_tr(j, 1, g),
                    lambda g, j=j: half_kq(j, "k", g),
                    lambda g, j=j: half_kq(j, "q", g),
                    lambda g, j=j: half_v(j, g)]
    # chunk c is emitted at k-tile 1 + 5c/6: half j's last chunk lands at
    # k-tile ~1+(5(j-2)+4)*5/6 < 4j-1, its first-use deadline
    for c in range(len(chunk_q)):
        chunk_at.setdefault(1 + (c * 5) // 6, []).append(chunk_q[c])

    # ---- stage C: attention (+ incremental output projection) -----------
    # load Wo up front so the per-qc partial output projection can overlap
    # the next query chunk's attention
    wo_sb = []
    for hl in range(2):
        st = stg.tile([64, D], F32, tag="wostg")
        nc.sync.dma_start(out=st[:], in_=wop[hl * 64:(hl + 1) * 64, :])
        woh = w_pool.tile([64, D], DTM, tag=f"wo{hl}")
        nc.vector.tensor_copy(out=woh[:], in_=st[:])
        wo_sb.append(woh)
    ot0 = ot_pool.tile([64, S], DTM, tag="OT")
    ot1 = ot_pool.tile([64, S], DTM, tag="OT")

    # Per-qc normalize + output-projection PE work is DEFERRED into the
    # NEXT qc's score stream (the PE executes its queue in order, so any
    # instruction waiting on the DVE reciprocal would otherwise stall the
    # whole pipeline at every qc boundary).
    deferred = []  # stage closures for the previous qc
    b_hist = []    # score-pair gate instructions, across qcs

    def make_stages(qc, osb0, osb1, rc0, rc1):
        qsl = slice(qc * 512, (qc + 1) * 512)

        def pin(i, gate):
            # the Tile scheduler reorders per-engine streams; without this
            # edge it hoists deferred PE work back to the qc boundary where
            # it stalls on the DVE normalize chain
            if gate is not None:
                _add_dep_helper(i.ins, gate.ins, sync=False,
                                reason="defer past boundary")

        def s1_norm(gate):
            # broadcast each head's reciprocal denominator row down 64
            # partitions, then scale the raw attention outputs into ot*.
            bct = psum1024()
            pin(mm(bct[0:64, 0:512], ones64_sb[64:65, :], rc0[64:65, :]), gate)
            mm(bct[0:64, 512:1024], ones64_sb[64:65, :], rc1[64:65, :])
            nc.vector.tensor_mul(ot0[:, qsl], osb0[0:64, :], bct[0:64, 0:512])
            nc.vector.tensor_mul(ot1[:, qsl], osb1[0:64, :], bct[0:64, 512:1024])

        def make_op(qp):
            def s_op(gate):
                # per-s-tile groups: 2 matmuls -> [128,512] copy -> DMA,
                # so the tail chain and the PE stream stay fine-grained
                ps = psum1024()
                for jj in range(2):
                    qt_i = qc * 4 + qp * 2 + jj
                    jsl = slice(jj * 512, (jj + 1) * 512)
                    pin(mm(ps[:, jsl], ot0[:, qt_i * P:(qt_i + 1) * P],
                           wo_sb[0][:], start=True, stop=False), gate)
                    mm(ps[:, jsl], ot1[:, qt_i * P:(qt_i + 1) * P],
                       wo_sb[1][:], start=False, stop=True)
                    ysb = y_pool.tile([P, 512], F32, tag="y")
                    nc.vector.tensor_copy(out=ysb[:], in_=ps[:, jsl])
                    nc.sync.dma_start(
                        out=out[qt_i * P:(qt_i + 1) * P, :], in_=ysb[:])
            return s_op

        return [s1_norm, make_op(0), make_op(1)]

    class QcState:
        def __init__(self, qc):
            self.qc = qc
            self.o0 = o_pool.tile([65, 512], F32, tag="O")
            self.o1 = o_pool.tile([65, 512], F32, tag="O")
            self.qq = qth[qc]
            self.pending = []  # [(pair, eat), ...] not yet AV-emitted
            self.eat = None

    def emit_av(st, pair, eat, gate):
        # fp8e4 DoubleRow: one matmul consumes the k-tile PAIR (2 fp8
        # weights per PE cell), streaming 2 rhs columns per cycle
        fl = dict(start=(pair == 0), stop=(pair == NT_S // 2 - 1))
        eav = eat[:, :].rearrange("p (ko h q) -> p ko h q", ko=2, h=2)
        i0 = mm(st.o0[:], v_lhsT(pair, 0), eav[:, :, 0, :],
                perf_mode=DR, **fl)
        i1 = mm(st.o1[:], v_lhsT(pair, 1), eav[:, :, 1, :],
                perf_mode=DR, **fl)
        if gate is not None:
            # order A@V after the next score pair: keeps the paired
            # heads adjacent in the PE stream
            _add_dep_helper(i0.ins, gate.ins, sync=False,
                            reason="attn pipeline order")
            _add_dep_helper(i1.ins, gate.ins, sync=False,
                            reason="attn pipeline order")

    def kt_step(st, ktile):
        kq = kth[ktile // 4]
        klo = (ktile % 4) * P
        ksl = slice(klo, klo + P)
        # both heads' scores share one [128,1024] PSUM tile
        sp = psum1024()
        a = mm(sp[:, 0:512], kq[0:64, ksl], st.qq[0:64, 0:SH])
        b = mm(sp[:, 512:1024], kq[64:128, ksl], st.qq[64:128, 0:SH])
        b_hist.append(b)
        # pin h64 right after h0: the pair streams through disjoint
        # PE row strips concurrently
        _add_dep_helper(b.ins, a.ins, sync=False, reason="pair order")
        # A@V lags two k-tile pairs behind the scores so its exp()
        # inputs are always long done.
        if len(st.pending) >= 2:
            ppr, pea = st.pending.pop(0)
            emit_av(st, ppr, pea, b)
        if ktile % 2 == 0:
            st.eat = e_pool.tile([P, 2048], F8, tag="ea")
        # exp straight to fp8e4 in the DoubleRow pair layout
        # [ko=parity, h, q]
        par = ktile % 2
        nc.scalar.activation(
            st.eat[:, par * 1024:(par + 1) * 1024], sp[:],
            EXP, scale=0.125)
        if par == 1:
            st.pending.append((ktile // 2, st.eat))

    def finish_qc(st):
        for ppr, pea in st.pending:
            emit_av(st, ppr, pea, None)
        # copy O out of PSUM immediately (frees the accumulator banks for
        # the next qc), take cheap reciprocals of the denominator rows;
        # the broadcast + scale + projection run via `deferred`.
        osb0 = rc_pool.tile([65, 512], F32, tag="osb")
        nc.vector.tensor_copy(out=osb0[:], in_=st.o0[:])
        osb1 = rc_pool.tile([65, 512], F32, tag="osb")
        nc.vector.tensor_copy(out=osb1[:], in_=st.o1[:])
        # reciprocal_approx_fast needs a partition-0-aligned multi-row AP
        # (a [1,512]@p64 slice returns garbage — measured); running it
        # over the whole tile costs the same (free-dim-bound) and only
        # row 64 (the denominators) is ever read.
        rc0 = rc_pool.tile([65, 512], F32, tag="rc")
        nc.vector.reciprocal_approx_fast(out=rc0[:], in_=osb0[:])
        rc1 = rc_pool.tile([65, 512], F32, tag="rc")
        nc.vector.reciprocal_approx_fast(out=rc1[:], in_=osb1[:])
        # f16 copies so the broadcast matmuls run at 1 cyc/row (f32 is 4)
        rch0 = rc_pool.tile([65, 512], F16, tag="rch")
        nc.vector.tensor_copy(out=rch0[64:65, :], in_=rc0[64:65, :])
        rch1 = rc_pool.tile([65, 512], F16, tag="rch")
        nc.vector.tensor_copy(out=rch1[64:65, :], in_=rc1[64:65, :])
        deferred.extend(make_stages(st.qc, osb0, osb1, rch0, rch1))

    def gate2():
        return b_hist[-2] if len(b_hist) >= 2 else None

    # qc0 and qc1 run INTERLEAVED k-tile by k-tile: the stage-A+B chunks
    # then have twice the wall-clock window before each deadline, so the
    # PE-bound prologue no longer starves the scalar engine.
    s0, s1 = QcState(0), QcState(1)
    for ktile in range(NT_S):
        for fn in chunk_at.get(ktile, ()):
            fn(gate2())
        kt_step(s0, ktile)
        kt_step(s1, ktile)
    finish_qc(s0)
    finish_qc(s1)
    for qc in range(2, QC):
        st = QcState(qc)
        for ktile in range(NT_S):
            # deferred normalize/out-proj of earlier qcs fills the hole
            # where the PE would idle waiting on exp to free a PSUM slot
            if deferred and ktile in (4, 7, 10, 13, 16, 19):
                deferred.pop(0)(gate2())
            kt_step(st, ktile)
        finish_qc(st)
    for fn in deferred:
        fn(None)


def build():
    nc = bacc.Bacc("TRN2", target_bir_lowering=False, debug=False,
                   num_devices=N_CORES)
    io = {}
    for nm, shape in (("xb", [S, D]), ("wqp", [D, P]), ("wkp", [D, P]),
                      ("wvp", [D, P]), ("wop", [P, D]), ("bqp", [P, 1]),
                      ("bkp", [P, 1]), ("bvp", [1, P]), ("ident", [P, P])):
        io[nm] = nc.dram_tensor(nm, shape, F32, kind="ExternalInput").ap()
    io["out"] = nc.dram_tensor("out", [S, D], F32, kind="ExternalOutput").ap()
    with tile.TileContext(nc) as tc:
        with ExitStack() as ctx:
            _emit(ctx, tc, io)
    nc.compile()
    return nc


def make_in_maps(inputs):
    f = lambda a: np.ascontiguousarray(np.asarray(a, dtype=np.float32))
    x = f(inputs["x"])
    Wq, Wk, Wv, Wo = (f(inputs[k]) for k in ("Wq", "Wk", "Wv", "Wo"))
    bq, bk, bv = (f(inputs[k]).reshape(-1) for k in ("bq", "bk", "bv"))
    ident = np.eye(P, dtype=np.float32)
    in_maps = []
    for c in range(N_CORES):
        b, pr = c // 4, c % 4
        cs = slice(pr * P, (pr + 1) * P)
        in_maps.append({
            "xb": x[b],
            "wqp": f(Wq[:, cs]), "wkp": f(Wk[:, cs]), "wvp": f(Wv[:, cs]),
            "wop": f(Wo[cs, :]),
            "bqp": f(bq[cs]).reshape(P, 1), "bkp": f(bk[cs]).reshape(P, 1),
            "bvp": f(bv[cs]).reshape(1, P),
            "ident": ident,
        })
    return in_maps


_CACHE = {}
LAST_EXEC_NS = None


def run(inputs, trace=False):
    global LAST_EXEC_NS
    if "nc" not in _CACHE:
        _CACHE["nc"] = build()
    nc = _CACHE["nc"]
    kw = {}
    if trace:
        import sys, types
        if "antenv.axon_hooks" not in sys.modules:
            sys.path.insert(0, "/root/.axon_site")
            try:
                from trn_agent_boot.trn_boot import _ntff_profile_via_ctypes
                hook = _ntff_profile_via_ctypes("/opt/axon/libaxon_pjrt.so")
                mod = types.ModuleType("antenv.axon_hooks")
                mod.get_axon_ntff_profile_hook = lambda: hook
                mod.set_axon_ntff_profile_hook = lambda h: None
                sys.modules["antenv.axon_hooks"] = mod
            except Exception:
                pass
        kw = dict(trace=True, trace_cores=[0])
    res = run_bass_kernel_spmd(nc, make_in_maps(inputs),
                               core_ids=list(range(N_CORES)), **kw)
    if trace:
        LAST_EXEC_NS = res.exec_time_ns
    bo = np.asarray(inputs["bo"], np.float32).reshape(1, D)
    out = np.empty((B, S, D), np.float32)
    for b in range(B):
        acc = res.results[b * 4][ "out"].astype(np.float32).copy()
        for pr in range(1, 4):
            acc += res.results[b * 4 + pr]["out"]
        out[b] = acc + bo
    return out


def kernel(**inputs) -> np.ndarray:
    return run(inputs, trace=False)



# revision 50
# speedup vs baseline: 1.3799x; 1.3799x over previous
"""Multi-head self-attention Trainium2 Bass kernel (8-core SPMD).

Sharding: tensor-parallel over (batch, head-pair). With B=2 batches and
H=8 heads there are exactly 8 (batch, head-pair) units; core c handles
batch c//4 and heads {2*(c%4), 2*(c%4)+1}. Each core computes Q/K/V for its
two heads over the full sequence, runs attention, and produces the partial
output projection O_pair @ Wo_pair (no bias). The host sums the four
partials per batch and adds the output bias — a cheap numpy reduction.
Per-core weight slices are passed as separate inputs so the program stays
SPMD-uniform.

Layout strategy: activations live transposed in SBUF ([D, S], d on
partitions). Projections then need no weight transposes:
  K^T = Wk^T x^T   (lhsT = Wk chunk, rhs = x^T chunk)
  V   = x Wv       (lhsT = x^T chunk, rhs = Wv chunk)
Scores are computed transposed ([k, q], k on partitions) so softmax's
denominator comes from a ones-column appended to V (row 64 of the attention
output accumulator), and A^T is directly consumable by the A@V matmul.
exp() runs on the scalar engine with the 1/sqrt(dk) folded into its scale.
The normalized per-head outputs O^T are exactly the lhsT the output
projection wants, so no transposes are needed anywhere except on the input x.

Matmul operands are fp16 except A@V, which runs in fp8e4 with
perf_mode=DoubleRow: exp() writes E directly as e4m3 in a k-tile-pair
layout and each A@V matmul consumes the pair at 2 fp8 MACs per PE cell
(measured end-to-end absmax relative error ~9e-3 vs the 2e-2 gate). All
accumulation is fp32 in PSUM.

Scheduling is built around two facts about the machine: engines execute
their queues in order, and the scalar engine's exp() stream (~266us
busy) is the pacing resource. So (1) the per-qc normalize/out-projection
work is deferred into the NEXT qc's k-tile loop and pinned there with
scheduler dependency edges — at the point where the PE would idle
waiting for exp to free a score PSUM slot; (2) the projections of
sequence halves 2-7 are prefetched (DMA + DVE f16 cast) and emitted as
~1us chunks paced through qc0's k-tile loop the same way; (3) the two
heads' score matmuls are row-tiled (tile_position row strips 0-63 /
64-127) so each k-tile's pair streams concurrently.
"""

from contextlib import ExitStack

import numpy as np

import concourse.bass as bass
import concourse.tile as tile
from concourse import bacc, mybir
from concourse.bass import _add_dep_helper
from concourse.bass_utils import run_bass_kernel_spmd

N_CORES = 8
B, S, D, H, DK = 2, 4096, 512, 8, 64
P = 128
NT_S = S // P                  # 32 sequence tiles
NT_D = D // P                  # 4 d-model chunks
QC = S // 512                  # 8 query chunks of 512
VW = 2 * 65                    # 130: per-k-tile width of the augmented V
F32 = mybir.dt.float32
F32R = mybir.dt.float32r
F16 = mybir.dt.float16
F8 = mybir.dt.float8e4
DR = mybir.MatmulPerfMode.DoubleRow
EXP = mybir.ActivationFunctionType.Exp

# "f16" (10 mantissa bits, 2.4 GHz MAC path + FWL), "f32r" (13 bits but
# pinned at the 1.2 GHz throttled clock), "f32" (exact, 4 cycles/row).
MM_DTYPE = "f16"
DTM = {"f32r": F32R, "f16": F16, "f32": F32}[MM_DTYPE]

# exp(x) ~= C[0]x^5 + ... on [-2.5, 2.5] (least-squares at Chebyshev
# nodes; logits are N(0, 0.33), observed |max| ~1.9). Used by the DVE
# polynomial-exp offload; abs err ~0.018 in f16 — far below the fp8e4
# quantization the result is stored at.
EC5, EC4, EC3, EC2, EC1, EC0 = (0.010762342, 0.056516835, 0.15879715,
                                0.46371029, 1.0062593, 1.0128646)
MULT = mybir.AluOpType.mult
ADD = mybir.AluOpType.add
# which k-tiles of each qc run exp on DVE instead of the scalar engine.
# Measured: DVE runs these ops at 1x (no 2x/4x modes engage) so the
# 7-pass polynomial costs ~7.5us per tile vs ACT's 1.14us — offload is a
# net loss; kept as an empty hook.
DVE_EXP_KT = {_q: () for _q in range(8)}


def _emit(ctx: ExitStack, tc: tile.TileContext, io: dict):
    nc = tc.nc
    xb = io["xb"]
    wqp, wkp, wvp, wop = io["wqp"], io["wkp"], io["wvp"], io["wop"]
    bqp, bkp, bvp = io["bqp"], io["bkp"], io["bvp"]
    ident = io["ident"]
    out = io["out"]

    mm = nc.tensor.matmul

    # ---- pools ------------------------------------------------------------
    consts = ctx.enter_context(tc.tile_pool(name="consts", bufs=1))
    xt_pool = ctx.enter_context(tc.tile_pool(name="xt", bufs=1))
    qt_pool = ctx.enter_context(tc.tile_pool(name="qt", bufs=1))
    kt_pool = ctx.enter_context(tc.tile_pool(name="kt", bufs=1))
    v_pool = ctx.enter_context(tc.tile_pool(name="v", bufs=1))
    ot_pool = ctx.enter_context(tc.tile_pool(name="ot", bufs=2))
    w_pool = ctx.enter_context(tc.tile_pool(name="w", bufs=1))
    stg = ctx.enter_context(tc.tile_pool(name="stg", bufs=3))
    e_pool = ctx.enter_context(tc.tile_pool(name="e", bufs=8))
    rc_pool = ctx.enter_context(tc.tile_pool(name="rc", bufs=4))
    y_pool = ctx.enter_context(tc.tile_pool(name="y", bufs=3))
    # PSUM (8 banks): shared scores/normalize/out-proj ring [128,1024]x3
    # = 6 banks + attention accumulators [65,512]x2 = 2 banks. The ring
    # of 3 gives the score stream one extra tile of exp lookahead; the
    # few normalize/proj allocs per qc ride the same ring (safe now that
    # they are deferred+pinned past the qc boundary).
    ps_pool = ctx.enter_context(tc.tile_pool(name="ps", bufs=3, space="PSUM"))
    o_pool = ctx.enter_context(tc.tile_pool(name="o", bufs=2, space="PSUM"))

    def psum1024(dt=F32):
        return ps_pool.tile([P, 1024], dt, tag="ps", name="ps")

    def psum512(dt=F32):
        return psum1024(dt)[:, 0:512]

    # ---- constants --------------------------------------------------------
    ident_sb = consts.tile([P, P], F32, tag="ident")
    nc.scalar.dma_start(out=ident_sb[:], in_=ident[:])
    ident16 = consts.tile([P, P], F16, tag="ident16")
    nc.vector.tensor_copy(out=ident16[:], in_=ident_sb[:])
    ones_f32 = consts.tile([P, 1], F32, tag="ones_f32")
    nc.vector.memset(ones_f32[:], 1.0)
    ones_sb = consts.tile([1, P], DTM, tag="ones")
    nc.vector.tensor_copy(out=ones_sb[:], in_=ones_f32[0:1, 0:1].broadcast_to([1, P]))
    # a f16 ones row living on partition 64 (denominator broadcast lhsT)
    ones64_sb = consts.tile([65, 64], F16, tag="ones64")
    nc.vector.memset(ones64_sb[64:65, :], 1.0)
    # per-partition bias columns for K^T/Q^T (fused into the PSUM->SBUF
    # copies); bv as a [1, 128] row for the rank-1 bias matmul.
    bkT = consts.tile([P, 1], F32, tag="bkT")
    nc.scalar.dma_start(out=bkT[:], in_=bkp[:])
    bqT = consts.tile([P, 1], F32, tag="bqT")
    nc.scalar.dma_start(out=bqT[:], in_=bqp[:])
    bv_st = consts.tile([1, P], F32, tag="bv_st")
    nc.scalar.dma_start(out=bv_st[:], in_=bvp[:])
    bv_sb = consts.tile([1, P], DTM, tag="bv")
    nc.vector.tensor_copy(out=bv_sb[:], in_=bv_st[:])

    # per-core weight slices -> fp16 SBUF tiles
    def load_w(ap, rows, cols, tag):
        st = stg.tile([P, (rows // P) * cols], F32, tag="wstg")
        nc.scalar.dma_start(
            out=st[:, :].rearrange("p (dc m) -> p dc m", dc=rows // P),
            in_=ap.rearrange("(dc p) m -> p dc m", p=P),
        )
        t = w_pool.tile([P, (rows // P) * cols], DTM, tag=tag)
        nc.vector.tensor_copy(out=t[:], in_=st[:])
        return t

    # x^T, Q^T, K^T, V are held at sequence-HALF-CHUNK granularity (8
    # tiles of 512 sequence positions each) so dependency tracking lets
    # attention start as soon as the first 512-chunk of K/Q/V exists, and
    # the remaining projection work streams into qc0's slack in ~1us
    # chunks without ever starving the scalar engine.
    SH = 512                    # sequence columns per half-chunk
    xTh = [xt_pool.tile([P, NT_D * SH], DTM, tag="xT", name=f"xT{j}",
                        bufs=8) for j in range(8)]

    def xslice(dc, s0, s1):
        j = s0 // SH
        return xTh[j][:, dc * SH + s0 - j * SH: dc * SH + s1 - j * SH]

    # ---- stages A+B, emitted as fine-grained chunks ----------------------
    wsb = {}
    qth = [qt_pool.tile([P, SH], DTM, tag="QT", name=f"QT{j}", bufs=8)
           for j in range(8)]
    kth = [kt_pool.tile([P, SH], DTM, tag="KT", name=f"KT{j}", bufs=8)
           for j in range(8)]
    # V in fp8e4, packed for DoubleRow A@V: per k-tile PAIR pr and head h,
    # lhsT cols pr%2*320 + h*160 + (ko*80 + j) with ko in {0,1} the two
    # k-tiles of the pair, j<65 (64 V dims + ones column), 15 cols pad so
    # the Ko step (80) is 16-aligned as DoubleRow requires.
    vqh = [v_pool.tile([P, 2 * 320], F8, tag="vaug", name=f"vq{j}", bufs=8)
           for j in range(8)]

    def v_lhsT(pair, h):
        base = (pair % 2) * 320 + h * 160
        return vqh[pair // 2][:, base:base + 160].rearrange(
            "p (ko w) -> p ko w", ko=2)[:, :, 0:65]
    xn_pool = ctx.enter_context(tc.tile_pool(name="xn", bufs=8))
    xh_pool = ctx.enter_context(tc.tile_pool(name="xh", bufs=28))
    xp_pool = ctx.enter_context(tc.tile_pool(name="xp", bufs=4))
    xh_all = {}  # st -> prefetched f16 x tile (quarters 1-3)

    def emit_exp_dve(sp, eat, parity):
        # exp via degree-5 polynomial on the vector engine: one
        # psum-read/scale pass, Horner-style chain in f16 (the
        # scalar_tensor_tensor form (p+c)*x carries no constant term, so
        # the power coefficients map onto the chain constants directly),
        # final +C0 pass writing the fp8 pair-layout slice.
        x16 = xp_pool.tile([P, 1024], F16, tag="x16")
        nc.vector.tensor_scalar(out=x16[:], in0=sp[:], scalar1=0.125,
                                scalar2=None, op0=MULT)
        p = xp_pool.tile([P, 1024], F16, tag="pp")
        nc.vector.tensor_scalar(out=p[:], in0=x16[:], scalar1=EC5,
                                scalar2=EC4, op0=MULT, op1=ADD)
        for g in (0.0, EC3, EC2, EC1):
            p2 = xp_pool.tile([P, 1024], F16, tag="pp")
            nc.vector.scalar_tensor_tensor(out=p2[:], in0=p[:], scalar=g,
                                           in1=x16[:], op0=ADD, op1=MULT)
            p = p2
        nc.vector.tensor_scalar(
            out=eat[:, parity * 1024:(parity + 1) * 1024], in0=p[:],
            scalar1=EC0, scalar2=None, op0=ADD)

    def prefetch_x(j):
        # DMA + f32->f16 cast well ahead of the half-chunk's compute, so
        # the PE work inserted into qc0's score stream never waits on the
        # memory chain. Casts go on DVE: gpsimd CAST measures ~1.9us per
        # tile (3.6x the cost model), DVE does it in ~0.55us.
        for st in range(4 * j, 4 * j + 4):
            xn = xn_pool.tile([P, D], F32, tag="xn")
            nc.sync.dma_start(out=xn[:], in_=xb[st * P:(st + 1) * P, :])
            xh = xh_pool.tile([P, D], F16, tag="xh")
            nc.vector.tensor_copy(out=xh[:], in_=xn[:])
            xh_all[st] = xh

    def pin_first(ins_list, gate):
        if gate is not None and ins_list:
            _add_dep_helper(ins_list[0].ins, gate.ins, sync=False,
                            reason="chunk after scores")

    def half_tr(j, ts, gate=None):
        # transpose x s-tiles 4j+2ts, 4j+2ts+1 into xTh[j]
        first = []
        for st in (4 * j + 2 * ts, 4 * j + 2 * ts + 1):
            if j < 1:
                # prologue: the PE is idle while the first x tiles stream
                # in, so burn the cheap-to-hide f32 transpose (no cast in
                # the latency chain)
                xn = xn_pool.tile([P, D], F32, tag="xn")
                nc.sync.dma_start(out=xn[:], in_=xb[st * P:(st + 1) * P, :])
                tp = psum1024()
                for dc in range(NT_D):
                    nc.tensor.transpose(
                        tp[:, dc * P:(dc + 1) * P],
                        xn[:, dc * P:(dc + 1) * P],
                        ident_sb[:],
                    )
            else:
                # f16 transpose runs at 1 cyc/row + FWL (f32: 2 cyc, none)
                xh = xh_all.pop(st)
                tp = psum1024(F16)
                for dc in range(NT_D):
                    t_i = nc.tensor.transpose(
                        tp[:, dc * P:(dc + 1) * P],
                        xh[:, dc * P:(dc + 1) * P],
                        ident16[:],
                    )
                    if not first:
                        first.append(t_i)
                        pin_first(first, gate)
            dst_ap = xTh[j][:, :].rearrange("p (dc s) -> p dc s", dc=NT_D)
            so = (st % 4) * P
            nc.vector.tensor_copy(
                out=dst_ap[:, :, so:so + P],
                in_=tp[:, 0:512].rearrange("p (dc j) -> p dc j", dc=NT_D),
            )


    def half_kq(j, which, gate=None):
        w_sb, dst, bT = ((wsb["wk"], kth[j], bkT) if which == "k" else
                         (wsb["wq"], qth[j], bqT))
        ps = psum1024()
        for dc in range(NT_D):
            m_i = mm(ps[:, 0:512], w_sb[:, dc * P:(dc + 1) * P],
                     xslice(dc, j * SH, (j + 1) * SH),
                     start=(dc == 0), stop=(dc == NT_D - 1))
            if dc == 0:
                pin_first([m_i], gate)
        nc.vector.tensor_scalar_add(out=dst[:, :], in0=ps[:, 0:512],
                                    scalar1=bT[:])

    def half_v(j, gate=None):
        nc.vector.tensor_copy(
            out=vqh[j][:, :].rearrange("p (pr h ko w) -> p pr h ko w",
                                       pr=2, h=2, ko=2)[:, :, :, :, 64:65],
            in_=ones_f32[:, 0:1].broadcast_to([P, 2, 2, 2, 1]),
        )
        first = []
        for pr in (2 * j, 2 * j + 1):
            # two V s-tiles (= one DoubleRow k-tile pair) per [128,1024]
            # tile (banks 0 and 1)
            ps = psum1024()
            for jj in range(2):
                st = 2 * pr + jj
                for dc in range(NT_D):
                    m_i = mm(ps[:, jj * 512:jj * 512 + P],
                             xslice(dc, st * P, (st + 1) * P),
                             wsb["wv"][:, dc * P:(dc + 1) * P],
                             start=(dc == 0), stop=False)
                    if not first:
                        first.append(m_i)
                        pin_first(first, gate)
                mm(ps[:, jj * 512:jj * 512 + P], ones_sb[0:1, :],
                   bv_sb[0:1, :], start=False, stop=True)
            dst = vqh[j][:, (pr % 2) * 320:(pr % 2 + 1) * 320]
            dst = dst.rearrange("p (h ko w) -> p h ko w", h=2, ko=2)[:, :, :, 0:64]
            src = ps[:, :].rearrange("p (ko r) -> p ko r", ko=2)[:, :, 0:P]
            nc.vector.tensor_copy(
                out=dst, in_=src.rearrange("p ko (h e) -> p h ko e", h=2)
            )

    # halves 0-1 (k-tiles 0-7) up front; halves 2-7 as ~1us chunks paced
    # through qc0's k-tile loop, each finishing before the k-tiles that
    # need it. Weight DMAs are interleaved after the x tiles they'd
    # otherwise delay; each is needed only once its projection starts.
    half_tr(0, 0)
    wsb["wk"] = load_w(wkp, D, P, "wk")
    half_tr(0, 1)
    wsb["wq"] = load_w(wqp, D, P, "wq")
    half_kq(0, "k")
    wsb["wv"] = load_w(wvp, D, P, "wv")
    half_kq(0, "q")
    half_v(0)
    for j in range(1, 8):
        prefetch_x(j)
    # halves 1-7 stream through qc0's k-tile loop; half 1 is front-loaded
    # (k-tiles 4-7 need it) so the first scores are not queued behind it
    chunk_q = [lambda g: half_tr(1, 0, g),
               lambda g: half_tr(1, 1, g),
               lambda g: half_kq(1, "k", g),
               lambda g: half_kq(1, "q", g),
               lambda g: half_v(1, g)]
    chunk_at = {0: [chunk_q[0]], 1: [chunk_q[1], chunk_q[2]],
                2: [chunk_q[3]], 3: [chunk_q[4]]}
    chunk_q = []
    for j in range(2, 8):
        chunk_q += [lambda g, j=j: half_tr(j, 0, g),
                    lambda g, j=j: half_tr(j, 1, g),
                    lambda g, j=j: half_kq(j, "k", g),
                    lambda g, j=j: half_kq(j, "q", g),
                    lambda g, j=j: half_v(j, g)]
    # chunk c is emitted at k-tile 1 + 5c/6: half j's last chunk lands at
    # k-tile ~1+(5(j-2)+4)*5/6 < 4j-1, its first-use deadline
    for c in range(len(chunk_q)):
        chunk_at.setdefault(1 + (c * 5) // 6, []).append(chunk_q[c])

    # ---- stage C: attention (+ incremental output projection) -----------
    # load Wo up front so the per-qc partial output projection can overlap
    # the next query chunk's attention
    wo_sb = []
    for hl in range(2):
        st = stg.tile([64, D], F32, tag="wostg")
        nc.scalar.dma_start(out=st[:], in_=wop[hl * 64:(hl + 1) * 64, :])
        woh = w_pool.tile([64, D], DTM, tag=f"wo{hl}")
        nc.vector.tensor_copy(out=woh[:], in_=st[:])
        wo_sb.append(woh)
    ot0 = ot_pool.tile([64, S], DTM, tag="OT")
    ot1 = ot_pool.tile([64, S], DTM, tag="OT")

    # Per-qc normalize + output-projection PE work is DEFERRED into the
    # NEXT qc's score stream (the PE executes its queue in order, so any
    # instruction waiting on the DVE reciprocal would otherwise stall the
    # whole pipeline at every qc boundary).
    deferred = []  # stage closures for the previous qc
    b_hist = []    # score-pair gate instructions, across qcs

    def make_stages(qc, osb0, osb1, rc0, rc1):
        qsl = slice(qc * 512, (qc + 1) * 512)

        def pin(i, gate):
            # the Tile scheduler reorders per-engine streams; without this
            # edge it hoists deferred PE work back to the qc boundary where
            # it stalls on the DVE normalize chain
            if gate is not None:
                _add_dep_helper(i.ins, gate.ins, sync=False,
                                reason="defer past boundary")

        def s1_norm(gate):
            # broadcast each head's reciprocal denominator row down 64
            # partitions, then scale the raw attention outputs into ot*.
            bct = psum1024()
            pin(mm(bct[0:64, 0:512], ones64_sb[64:65, :], rc0[64:65, :]), gate)
            mm(bct[0:64, 512:1024], ones64_sb[64:65, :], rc1[64:65, :])
            nc.vector.tensor_mul(ot0[:, qsl], osb0[0:64, :], bct[0:64, 0:512])
            nc.vector.tensor_mul(ot1[:, qsl], osb1[0:64, :], bct[0:64, 512:1024])

        def make_op(qp):
            def s_op(gate):
                # per-s-tile groups: 2 matmuls -> [128,512] copy -> DMA,
                # so the tail chain and the PE stream stay fine-grained
                ps = psum1024()
                for jj in range(2):
                    qt_i = qc * 4 + qp * 2 + jj
                    jsl = slice(jj * 512, (jj + 1) * 512)
                    pin(mm(ps[:, jsl], ot0[:, qt_i * P:(qt_i + 1) * P],
                           wo_sb[0][:], start=True, stop=False), gate)
                    mm(ps[:, jsl], ot1[:, qt_i * P:(qt_i + 1) * P],
                       wo_sb[1][:], start=False, stop=True)
                    ysb = y_pool.tile([P, 512], F32, tag="y")
                    nc.vector.tensor_copy(out=ysb[:], in_=ps[:, jsl])
                    nc.sync.dma_start(
                        out=out[qt_i * P:(qt_i + 1) * P, :], in_=ysb[:])
            return s_op

        return [s1_norm, make_op(0), make_op(1)]

    for qc in range(QC):
        o0 = o_pool.tile([65, 512], F32, tag="O")
        o1 = o_pool.tile([65, 512], F32, tag="O")

        def emit_av(pair, eat, gate):
            # fp8e4 DoubleRow: one matmul consumes the k-tile PAIR (2 fp8
            # weights per PE cell), streaming 2 rhs columns per cycle
            fl = dict(start=(pair == 0), stop=(pair == NT_S // 2 - 1))
            eav = eat[:, :].rearrange("p (ko h q) -> p ko h q",
                                      ko=2, h=2)
            i0 = mm(o0[:], v_lhsT(pair, 0), eav[:, :, 0, :],
                    perf_mode=DR, **fl)
            i1 = mm(o1[:], v_lhsT(pair, 1), eav[:, :, 1, :],
                    perf_mode=DR, **fl)
            if gate is not None:
                # order A@V after the next score pair: keeps the paired
                # heads adjacent in the PE stream
                _add_dep_helper(i0.ins, gate.ins, sync=False,
                                reason="attn pipeline order")
                _add_dep_helper(i1.ins, gate.ins, sync=False,
                                reason="attn pipeline order")

        qq = qth[qc]
        qls = slice(0, SH)
        pending = []  # [(pair, eat), ...] not yet AV-emitted
        eat = None
        for ktile in range(NT_S):
            # inserted work goes at the TOP of the iteration, gated two
            # k-tiles back: the PE's in-order queue idles right before
            # each score pair waiting on the exp that frees its PSUM
            # slot, and work placed here fills exactly that hole.
            gate2 = b_hist[-2] if len(b_hist) >= 2 else None
            if qc == 0:
                for fn in chunk_at.get(ktile, ()):
                    fn(gate2)
            if deferred and ktile in (8, 14, 20):
                deferred.pop(0)(gate2)
            kq = kth[ktile // 4]
            klo = (ktile % 4) * P
            ksl = slice(klo, klo + P)
            # both heads' scores share one [128,1024] PSUM tile
            sp = psum1024()
            a = mm(sp[:, 0:512], kq[0:64, ksl], qq[0:64, qls])
            b = mm(sp[:, 512:1024], kq[64:128, ksl], qq[64:128, qls])
            b_hist.append(b)
            # pin h64 right after h0: the pair streams through disjoint
            # PE row strips concurrently
            _add_dep_helper(b.ins, a.ins, sync=False, reason="pair order")
            # A@V lags two k-tile pairs behind the scores so its exp()
            # inputs are always long done.
            if len(pending) >= 2:
                ppr, pea = pending.pop(0)
                emit_av(ppr, pea, b)
            if ktile % 2 == 0:
                eat = e_pool.tile([P, 2048], F8, tag="ea")
            # exp straight to fp8e4 in the DoubleRow pair layout
            # [h, ko=parity, q]; a few k-tiles per qc run on DVE instead
            # to unload the pacing scalar engine
            if ktile in DVE_EXP_KT[qc]:
                emit_exp_dve(sp, eat, ktile % 2)
            else:
                par = ktile % 2
                nc.scalar.activation(
                    eat[:, par * 1024:(par + 1) * 1024], sp[:],
                    EXP, scale=0.125)
            if ktile % 2 == 1:
                pending.append((ktile // 2, eat))
        for ppr, pea in pending:
            emit_av(ppr, pea, None)
        # copy O out of PSUM immediately (frees the accumulator banks for
        # the next qc), take cheap [1,512] reciprocals of the denominator
        # rows; the broadcast + scale + projection run via `deferred`.
        osb0 = rc_pool.tile([65, 512], F32, tag="osb")
        nc.vector.tensor_copy(out=osb0[:], in_=o0[:])
        osb1 = rc_pool.tile([65, 512], F32, tag="osb")
        nc.vector.tensor_copy(out=osb1[:], in_=o1[:])
        # reciprocal_approx_fast needs a partition-0-aligned multi-row AP
        # (a [1,512]@p64 slice returns garbage — measured); running it over
        # the whole tile costs the same (free-dim-bound) and only row 64
        # (the denominators) is ever read.
        rc0 = rc_pool.tile([65, 512], F32, tag="rc")
        nc.vector.reciprocal_approx_fast(out=rc0[:], in_=osb0[:])
        rc1 = rc_pool.tile([65, 512], F32, tag="rc")
        nc.vector.reciprocal_approx_fast(out=rc1[:], in_=osb1[:])
        # f16 copies so the broadcast matmuls run at 1 cyc/row (f32 is 4)
        rch0 = rc_pool.tile([65, 512], F16, tag="rch")
        nc.vector.tensor_copy(out=rch0[64:65, :], in_=rc0[64:65, :])
        rch1 = rc_pool.tile([65, 512], F16, tag="rch")
        nc.vector.tensor_copy(out=rch1[64:65, :], in_=rc1[64:65, :])
        deferred.extend(make_stages(qc, osb0, osb1, rch0, rch1))
    for fn in deferred:
        fn(None)


def build():
    nc = bacc.Bacc("TRN2", target_bir_lowering=False, debug=False,
                   num_devices=N_CORES)
    io = {}
    for nm, shape in (("xb", [S, D]), ("wqp", [D, P]), ("wkp", [D, P]),
                      ("wvp", [D, P]), ("wop", [P, D]), ("bqp", [P, 1]),
                      ("bkp", [P, 1]), ("bvp", [1, P]), ("ident", [P, P])):
        io[nm] = nc.dram_tensor(nm, shape, F32, kind="ExternalInput").ap()
    io["out"] = nc.dram_tensor("out", [S, D], F32, kind="ExternalOutput").ap()
    with tile.TileContext(nc) as tc:
        with ExitStack() as ctx:
            _emit(ctx, tc, io)
    nc.compile()
    return nc


def make_in_maps(inputs):
    f = lambda a: np.ascontiguousarray(np.asarray(a, dtype=np.float32))
    x = f(inputs["x"])
    Wq, Wk, Wv, Wo = (f(inputs[k]) for k in ("Wq", "Wk", "Wv", "Wo"))
    bq, bk, bv = (f(inputs[k]).reshape(-1) for k in ("bq", "bk", "bv"))
    ident = np.eye(P, dtype=np.float32)
    in_maps = []
    for c in range(N_CORES):
        b, pr = c // 4, c % 4
        cs = slice(pr * P, (pr + 1) * P)
        in_maps.append({
            "xb": x[b],
            "wqp": f(Wq[:, cs]), "wkp": f(Wk[:, cs]), "wvp": f(Wv[:, cs]),
            "wop": f(Wo[cs, :]),
            "bqp": f(bq[cs]).reshape(P, 1), "bkp": f(bk[cs]).reshape(P, 1),
            "bvp": f(bv[cs]).reshape(1, P),
            "ident": ident,
        })
    return in_maps


_CACHE = {}
LAST_EXEC_NS = None


def run(inputs, trace=False):
    global LAST_EXEC_NS
    if "nc" not in _CACHE:
        _CACHE["nc"] = build()
    nc = _CACHE["nc"]
    kw = {}
    if trace:
        import sys, types
        if "antenv.axon_hooks" not in sys.modules:
            sys.path.insert(0, "/root/.axon_site")
            try:
                from trn_agent_boot.trn_boot import _ntff_profile_via_ctypes
                hook = _ntff_profile_via_ctypes("/opt/axon/libaxon_pjrt.so")
                mod = types.ModuleType("antenv.axon_hooks")
                mod.get_axon_ntff_profile_hook = lambda: hook
                mod.set_axon_ntff_profile_hook = lambda h: None
                sys.modules["antenv.axon_hooks"] = mod
            except Exception:
                pass
        kw = dict(trace=True, trace_cores=[0])
    res = run_bass_kernel_spmd(nc, make_in_maps(inputs),
                               core_ids=list(range(N_CORES)), **kw)
    if trace:
        LAST_EXEC_NS = res.exec_time_ns
    bo = np.asarray(inputs["bo"], np.float32).reshape(1, D)
    out = np.empty((B, S, D), np.float32)
    for b in range(B):
        acc = res.results[b * 4][ "out"].astype(np.float32).copy()
        for pr in range(1, 4):
            acc += res.results[b * 4 + pr]["out"]
        out[b] = acc + bo
    return out


def kernel(**inputs) -> np.ndarray:
    return run(inputs, trace=False)

